# revision 36
# baseline (speedup 1.0000x reference)
"""Guide-token attention kernel for Trainium2 (8 NeuronCores).

Module: y[b] = softmax(((Q+tQ) @ (K+tK)^T)/sqrt(hd)) @ V  per head, where
  Q = x @ Wq^T + bq, K = x @ Wk^T + bk, V = x @ Wv^T + bv,
  tQ/tK are projections of a per-batch guide token (broadcast over seq).

Shapes: x [4, 1024, 1024], tokens [4, 1, 1024], W* [1024, 1024], b* [1024].
H=16 heads, hd=64.

Sharding: 8 cores = 4 batches x 2 head-groups (8 heads each); weights
column-sharded per head group; each core sees one batch -> no cross-core
communication.

Design (v8, from v7 + trace-driven fixes):
  - PE is the bottleneck: 384 effective N=512 slots x 216 ns ~= 83 us.
    v7 trace: steady-state already hits 216 ns/slot; losses were (a) ACT
    head-of-line-blocked until 21.9us by input-DMA posts on the Scalar
    queue, (b) HAM cold until 20.3us, (c) end-phase AV backlog draining
    after ACT finished, with HAM re-throttle + psAV recycle blocked on the
    normalize chain, (d) 5.1us final normalize chain.
  - Input DMA posts now ride the Sync + GpSimd queues; the Scalar queue
    runs exps only (plus phase-0 evictions + early table load).
  - Half-pair score pipeline: one [128, 2, 512] PSUM tile (A_j B_j row
    pair) + one exp per kt, psA bufs=2 -> a true rolling double buffer.
    scores(N+1) overlap exp(N); the v7/v8.0 schemes allocated both bufs
    per pair, serializing exp -> scores -> exp (~0.9us/pair lost).
  - 28 N=128 warmup matmuls upfront (wrm memset on GpSimd before its DMA
    posts) bridge the preamble->DMA-ready window; HAM flips to 2.4 GHz at
    ~10.7us and the real MM stream starts warm instead of at 427 ns/MM.
  - AV pops start at u>=1, keep backlog ~2, and break after a p3 pop so
    the unit's normalize chain gets a pair-window before the next unit's
    p0 alloc needs the psAV banks.
  - Normalize: reciprocal straight from the PSUM den row, av copied
    PSUM->SBUF (frees the bank ~2.4us earlier), GpSimd partition-bcast,
    one DVE multiply, per-(hp,qb) 128KB output flush.
"""

import os

import numpy as np
import ml_dtypes

import concourse.bass as bass
import concourse.tile as tile
from concourse import bacc
from concourse import mybir
from concourse.bass_utils import run_bass_kernel_spmd

B = 4
S = 1024
D = 1024
H = 16
HD = 64
NCORES = 8
FPG = 512          # features per head-group (8 heads * 64)
NKC = D // 128     # contraction chunks for projections
NFT = FPG // 128   # feature tiles per group
NST = S // 128     # sequence tiles
NQB = S // 512     # 512-wide query blocks
HPG = 8            # heads per group
NPAIR = NST // 2   # kt pairs per unit

BF16 = mybir.dt.bfloat16
F32 = mybir.dt.float32

PAIR_BUDGET = 10   # non-score PE slots emitted per score pair
AVQ_CAP = 10       # force AV pops above this backlog (bounds probs pool)
AV_RESERVE = 2     # AV groups held back as PE anti-starvation reserve

_CACHE = {}


def _build():
    nc = bacc.Bacc()

    # ---- DRAM inputs ----
    # sync queue:   xA0 xB0 qadd kadd wk0 xA1 xB1 wk1       (+ yT out later)
    # gpsimd queue: wq0 xC0 xD0 wq1 xC1 xD1 wv wq2 wk2 wq3 wk3
    xd = {}
    for cname in ("xA0", "xB0", "xC0", "xD0", "xA1", "xB1", "xC1", "xD1"):
        xd[cname] = nc.declare_dram_parameter(cname, [128, 2, 512], BF16, isOutput=False)
    wqd = [nc.declare_dram_parameter(f"wq{f}", [128, NKC, 128], BF16, isOutput=False)
           for f in range(NFT)]
    # wk0 carries qadd+kadd (fp32 bit-packed into bf16 column pairs) in
    # the last 16 columns of each row — a standalone [128,4] fp32 transfer
    # is 128 16-byte descriptors and stalls the HW DGE ring for ~3us
    # (descriptor-rate-bound)
    wkd = [nc.declare_dram_parameter(
               f"wk{f}", [128, NKC * 128 + (16 if f == 0 else 0)], BF16,
               isOutput=False)
           for f in range(NFT)]
    wvd = nc.declare_dram_parameter("wv", [128, NKC, FPG], BF16, isOutput=False)
    yTd = nc.declare_dram_parameter("yT", [NFT, 128, S], BF16, isOutput=True)

    with tile.TileContext(nc) as tc:
        with (
            tc.tile_pool(name="persist", bufs=1) as persist,
            tc.tile_pool(name="probs", bufs=24) as probs_pool,
            tc.tile_pool(name="norm", bufs=4) as norm_pool,
            tc.tile_pool(name="avsb", bufs=4) as avsb_pool,
            tc.tile_pool(name="psP", bufs=2, space=bass.MemorySpace.PSUM) as psP,
            tc.tile_pool(name="psA", bufs=2, space=bass.MemorySpace.PSUM) as psA,
            tc.tile_pool(name="psAV", bufs=2, space=bass.MemorySpace.PSUM) as psAV,
        ):
            # ---- persistent SBUF tensors (chunked to DMA granularity) ----
            xts = {(c, h): persist.tile([128, 2, 512], BF16, name=f"xt{c}{h}")
                   for c in range(4) for h in range(2)}
            wq_sb = [persist.tile([128, NKC, 128], BF16, name=f"wqs{f}")
                     for f in range(NFT)]
            wk_sb = [persist.tile([128, NKC * 128 + (16 if f == 0 else 0)],
                                  BF16, name=f"wks{f}")
                     for f in range(NFT)]
            wv_sb = persist.tile([128, NKC, FPG], BF16)
            cq = [persist.tile([128, S], BF16, name=f"cq{i}") for i in range(NFT)]
            ck = [persist.tile([128, S], BF16, name=f"ck{i}") for i in range(NFT)]
            vts = [persist.tile([128, HPG, HD + 1], BF16, name=f"vt{i}")
                   for i in range(NST)]
            yt = persist.tile([128, NFT, S], BF16)
            wrm = persist.tile([128, 128], BF16)
            scr = persist.tile([1, 4], F32)

            # warmup source memset FIRST on the (otherwise idle) gpsimd
            # queue so warmup matmuls can start right after the preamble
            nc.gpsimd.memset(wrm[:], 0.0)
            nc.gpsimd.memset(scr[:], 0.0)

            # ---- input DMAs on the two HW DGE rings (SP + Act only — the
            # gpsimd ring is software-DGE: ~5us post-to-land latency and an
            # expensive teardown drain).  Scalar takes only the 5 earliest
            # posts (ring-fresh, done by ~10.5us) so exps never queue behind
            # a ring-flow-control wait; sync takes the rest.  ----
            nc.scalar.dma_start(out=wq_sb[0][:], in_=wqd[0][:])
            nc.scalar.dma_start(out=wk_sb[0][:], in_=wkd[0][:])
            nc.scalar.dma_start(out=xts[(2, 0)][:], in_=xd["xC0"][:])
            nc.scalar.dma_start(out=xts[(3, 0)][:], in_=xd["xD0"][:])
            nc.scalar.dma_start(out=wq_sb[1][:], in_=wqd[1][:])
            nc.sync.dma_start(out=xts[(0, 0)][:], in_=xd["xA0"][:])
            nc.sync.dma_start(out=xts[(1, 0)][:], in_=xd["xB0"][:])
            nc.sync.dma_start(out=xts[(0, 1)][:], in_=xd["xA1"][:])
            nc.sync.dma_start(out=xts[(1, 1)][:], in_=xd["xB1"][:])
            nc.sync.dma_start(out=wk_sb[1][:], in_=wkd[1][:])
            nc.sync.dma_start(out=wv_sb[:], in_=wvd[:])
            nc.sync.dma_start(out=xts[(2, 1)][:], in_=xd["xC1"][:])
            nc.sync.dma_start(out=xts[(3, 1)][:], in_=xd["xD1"][:])
            nc.sync.dma_start(out=wq_sb[2][:], in_=wqd[2][:])
            nc.sync.dma_start(out=wk_sb[2][:], in_=wkd[2][:])
            nc.sync.dma_start(out=wq_sb[3][:], in_=wqd[3][:])
            nc.sync.dma_start(out=wk_sb[3][:], in_=wkd[3][:])

            # ones columns for the AV denominator rows
            for st in range(NST):
                nc.vector.memset(vts[st][:, :, HD:HD + 1], 1.0)

            # trigger the ACT table load ASAP so the first exp doesn't pay
            # it; uses a dedicated scratch so warmup matmuls (which read
            # wrm) don't serialize behind this ACT op
            nc.scalar.activation(
                out=scr[0:1, 0:1], in_=scr[0:1, 0:1],
                func=mybir.ActivationFunctionType.Identity,
            )

            # ---- HAM warmup machinery (N=128: 8x cheaper queue time) ----
            wacc = psAV.tile([128, 512], F32, tag="psAV")
            warm_left = [52]

            def warm_mm():
                if warm_left[0] > 0:
                    warm_left[0] -= 1
                    nc.tensor.matmul(
                        wacc[:, 0:128], wrm[:], wrm[:], start=True, stop=True
                    )

            # bridge preamble-end (~7.4us) to first-chunk-landed (~12.3us:
            # HW-DGE ring start ~2.5us + ~1.1us per 256KB chunk) so the PE
            # never idles long enough to reset the HAM activity window —
            # phase-0 then runs entirely at 2.4 GHz
            for _ in range(38):
                warm_mm()

            # ---- projection building blocks ----
            v_done = [0]      # V groups fully emitted (gates AV emission)
            qk_done = set()   # (which, ft, sb) evictions emitted

            def qk_group(which, ft, sb, evict_on_act=False):
                """QT/KT [128 feat, 512 q] accumulated over D chunks, evicted
                to bf16 with the guide-token add (+1/8 scale folded into Q).
                Yields (slots, op)."""
                base = NKC * 128
                if which == "q":
                    scale, dst = 0.125, cq[ft]
                    w_col = lambda kc: wq_sb[ft][:, kc, :]
                    bias = wk_sb[0][:, base + 2 * ft:base + 2 * ft + 2]
                else:
                    scale, dst = 1.0, ck[ft]
                    w_col = lambda kc: wk_sb[ft][:, kc * 128:(kc + 1) * 128]
                    bias = wk_sb[0][:, base + 8 + 2 * ft:base + 8 + 2 * ft + 2]
                bias = bias.bitcast(F32)
                acc = psP.tile([128, 512], F32, tag="psP")
                for kc in range(NKC):
                    yield 1, lambda kc=kc, acc=acc: nc.tensor.matmul(
                        acc[:],
                        w_col(kc),
                        xts[(kc // 2, sb)][:, kc % 2, :],
                        start=(kc == 0),
                        stop=(kc == NKC - 1),
                    )

                def evict(acc=acc, bias=bias):
                    if evict_on_act:
                        # ScalarE is idle before its first exp: out =
                        # Identity(acc*scale + bias) == the same eviction,
                        # off the (busier) VectorE queue
                        nc.scalar.activation(
                            out=dst[:, sb * 512:(sb + 1) * 512],
                            in_=acc[:],
                            func=mybir.ActivationFunctionType.Identity,
                            bias=bias,
                            scale=scale,
                        )
                    else:
                        nc.vector.tensor_scalar(
                            out=dst[:, sb * 512:(sb + 1) * 512],
                            in0=acc[:],
                            scalar1=scale,
                            scalar2=bias,
                            op0=mybir.AluOpType.mult,
                            op1=mybir.AluOpType.add,
                        )
                    qk_done.add((which, ft, sb))

                yield 0, evict

            def v_group(st):
                """V [128 seq, 512 feat] natural layout, strided into vts."""
                acc = psP.tile([128, 512], F32, tag="psP")
                for kc in range(NKC):
                    yield 1, lambda kc=kc, acc=acc: nc.tensor.matmul(
                        acc[:],
                        xts[(kc // 2, st // 4)][:, kc % 2,
                                                (st % 4) * 128:(st % 4 + 1) * 128],
                        wv_sb[:, kc, :],
                        start=(kc == 0),
                        stop=(kc == NKC - 1),
                    )

                def evict(acc=acc):
                    nc.vector.tensor_copy(out=vts[st][:, :, 0:HD], in_=acc[:])
                    v_done[0] += 1

                yield 0, evict

            # keyed filler groups: budget flow follows FILLER_ORDER, but
            # pair-readiness drains pull exactly the group they need (groups
            # are self-contained, so out-of-order emission is dep-safe)
            FILLER_ORDER = (
                [("k", 0, 1), ("q", 0, 1), ("q", 1, 0), ("k", 1, 0),
                 ("k", 1, 1), ("v", 0), ("v", 1), ("v", 2), ("v", 3),
                 ("q", 1, 1)]
                + [("v", st) for st in range(4, NST)]
                + [("q", 2, 0), ("k", 2, 0), ("k", 2, 1), ("q", 2, 1),
                   ("q", 3, 0), ("k", 3, 0), ("k", 3, 1), ("q", 3, 1)]
            )
            fgens = {}
            for key in FILLER_ORDER:
                if key[0] == "v":
                    fgens[key] = v_group(key[1])
                else:
                    fgens[key] = qk_group(*key)

            def drain_group(key):
                g = fgens.get(key)
                if g is None:
                    return
                for _, op in g:
                    op()
                del fgens[key]

            def filler_stream():
                for key in FILLER_ORDER:
                    g = fgens.get(key)
                    if g is None:
                        continue
                    sentinel = object()
                    while True:
                        item = next(g, sentinel)
                        if item is sentinel:
                            break
                        yield item
                    if fgens.get(key) is g:
                        del fgens[key]

            # ---- AV + normalize ----
            av_tiles = {}

            def av_ops(u, p, pairs):
                hp, qb = UNITS[u]
                if p == 0:
                    av_tiles[u] = (
                        psAV.tile([HD + 1, 512], F32, tag="psAV", name=f"av{u}e"),
                        psAV.tile([HD + 1, 512], F32, tag="psAV", name=f"av{u}o"),
                    )
                av_e, av_o = av_tiles[u]
                last_u = u == len(UNITS) - 1
                dens = {}
                for j in range(2):
                    kt = 2 * p + j
                    prs = pairs[kt]
                    nc.tensor.matmul(
                        av_e[:], vts[kt][:, 2 * hp, :], prs[:, 0, :],
                        start=(kt == 0), stop=(kt == NST - 1),
                    )
                    if last_u and kt == NST - 1:
                        # den_e copy issues while the final o-head matmul
                        # still runs — shortens the tail chain
                        den_e = norm_pool.tile([1, 512], F32, tag="den",
                                               name="den_e_tail")
                        nc.vector.tensor_copy(
                            out=den_e[:], in_=av_e[HD:HD + 1, :])
                        dens[0] = den_e
                    nc.tensor.matmul(
                        av_o[:], vts[kt][:, 2 * hp + 1, :], prs[:, 1, :],
                        start=(kt == 0), stop=(kt == NST - 1),
                    )
                if p == NPAIR - 1:
                    qsl = slice(qb * 512, (qb + 1) * 512)
                    # note: custom-DVE ops (reciprocal_approx_fast) must
                    # read from base partition 0 on HW — the input partition
                    # offset is silently dropped (sim models it fine) — so
                    # the den row is first copied to partition 0 by a plain
                    # DVE copy.
                    if last_u:
                        # tail mode: nothing recycles these banks, so skip
                        # the av copies and multiply straight from PSUM;
                        # bcasts overlap the o-head's den/recip on DVE and
                        # the e-head's flush overlaps the o-head's multiply
                        recb_of = {}
                        for h_i, av in ((0, av_e), (1, av_o)):
                            den = dens.get(h_i)
                            if den is None:
                                den = norm_pool.tile([1, 512], F32, tag="den")
                                nc.vector.tensor_copy(
                                    out=den[:], in_=av[HD:HD + 1, :])
                            rec = norm_pool.tile([1, 512], F32, tag="rec")
                            nc.vector.reciprocal_approx_fast(
                                out=rec[:], in_=den[:])
                            recb = norm_pool.tile([HD, 512], F32, tag="recb")
                            nc.gpsimd.partition_broadcast(recb[:], rec[:])
                            recb_of[h_i] = recb
                        for h_i, av in ((0, av_e), (1, av_o)):
                            nc.vector.tensor_tensor(
                                out=yt[h_i * 64:h_i * 64 + 64, hp, qsl],
                                in0=av[0:HD, :],
                                in1=recb_of[h_i][:],
                                op=mybir.AluOpType.mult,
                            )
                            nc.sync.dma_start(
                                out=yTd[hp][h_i * 64:h_i * 64 + 64, qsl],
                                in_=yt[h_i * 64:h_i * 64 + 64, hp, qsl])
                    else:
                        for h_i, av in ((0, av_e), (1, av_o)):
                            # av copy FIRST: the psAV bank frees ~0.7us
                            # after the last AV matmul instead of after the
                            # whole normalize chain (kills the ~1.4us PE
                            # stall at every unit boundary)
                            sb_av = avsb_pool.tile([HD + 1, 512], F32,
                                                   tag="avsb")
                            nc.vector.tensor_copy(out=sb_av[:], in_=av[:])
                            den = norm_pool.tile([1, 512], F32, tag="den")
                            nc.vector.tensor_copy(
                                out=den[:], in_=sb_av[HD:HD + 1, :])
                            rec = norm_pool.tile([1, 512], F32, tag="rec")
                            nc.vector.reciprocal_approx_fast(
                                out=rec[:], in_=den[:])
                            recb = norm_pool.tile([HD, 512], F32, tag="recb")
                            nc.gpsimd.partition_broadcast(recb[:], rec[:])
                            nc.vector.tensor_tensor(
                                out=yt[h_i * 64:h_i * 64 + 64, hp, qsl],
                                in0=sb_av[0:HD, :],
                                in1=recb[:],
                                op=mybir.AluOpType.mult,
                            )
                        nc.sync.dma_start(out=yTd[hp][:, qsl],
                                          in_=yt[:, hp, qsl])
                    del av_tiles[u]

            # ---- phase 0: Q00 and K00 interleaved kc-wise so both track x
            # chunk arrival; warmups pad the DMA-bound stretch (HAM warm) ----
            g_q = qk_group("q", 0, 0, evict_on_act=True)
            g_k = qk_group("k", 0, 0, evict_on_act=True)
            for i in range(NKC + 1):
                for g in (g_q, g_k):
                    item = next(g, None)
                    if item is not None:
                        item[1]()
                # 4 warmups per kc step pad the x-chunk arrival gaps —
                # phase-0 is DMA-paced anyway, so these are near-free and
                # keep the HAM MID window from seeing idle
                for _ in range(4):
                    warm_mm()
            # cover the eviction->first-score-pair latency
            for _ in range(3):
                warm_mm()

            UNITS = [(hp, qb) for hp in range(HPG // 2) for qb in range(NQB)]
            fillers = filler_stream()
            fillers_done = [False]
            avq = []
            pairs_of = {}

            def next_filler():
                item = next(fillers, None)
                if item is None:
                    fillers_done[0] = True
                    return None
                return item

            def pop_av():
                """Emit the oldest pending AV group if allowed; 4 PE slots.
                Returns (slots, was_last_group_of_unit)."""
                if not avq:
                    return 0, False
                au, ap_ = avq[0]
                if v_done[0] < 2 * ap_ + 2:
                    return 0, False
                avq.pop(0)
                av_ops(au, ap_, pairs_of[au])
                return 4, ap_ == NPAIR - 1

            def pair_ready(hp, qb, p):
                return ("q", hp, qb) in qk_done and ("k", hp, p // 2) in qk_done

            def emit_half(u, p, j):
                """One kt's scores (A/B row-tiled pair) + one exp.  With psA
                bufs=2 this forms a rolling double buffer: scores for half
                N+1 overlap exp N on ACT."""
                hp, qb = UNITS[u]
                qsl = slice(qb * 512, (qb + 1) * 512)
                kt = 2 * p + j
                ksl = slice(kt * 128, (kt + 1) * 128)
                sct = psA.tile([128, 2, 512], F32, tag="psA")
                nc.tensor.matmul(
                    sct[:, 0, :], ck[hp][0:64, ksl], cq[hp][0:64, qsl],
                    start=True, stop=True,
                )
                nc.tensor.matmul(
                    sct[:, 1, :], ck[hp][64:128, ksl], cq[hp][64:128, qsl],
                    start=True, stop=True,
                )
                prs = probs_pool.tile([128, 2, 512], BF16, tag="probs")
                nc.scalar.activation(
                    out=prs[:], in_=sct[:],
                    func=mybir.ActivationFunctionType.Exp,
                )
                pairs_of[u][kt] = prs
                if j == 1:
                    avq.append((u, p))

            HALF_BUDGET = PAIR_BUDGET // 2
            for u in range(len(UNITS)):
                pairs_of[u] = {}
                last_u = u == len(UNITS) - 1
                for p in range(NPAIR):
                    # correctness: projections this pair reads must be
                    # in-stream before its score matmuls (targeted pull)
                    hp_, qb_ = UNITS[u]
                    if ("q", hp_, qb_) not in qk_done:
                        drain_group(("q", hp_, qb_))
                    if ("k", hp_, p // 2) not in qk_done:
                        drain_group(("k", hp_, p // 2))
                    emit_half(u, p, 0)
                    # fillers keep PE busy while exp(kt0) runs
                    budget = HALF_BUDGET
                    while budget > 0:
                        item = next_filler()
                        if item is None:
                            break
                        budget -= item[0]
                        item[1]()
                    emit_half(u, p, 1)
                    # rate-match ACT: AV pops + projection fillers; after a
                    # unit-final (p3) pop, stop popping so its normalize
                    # chain gets a window before the next unit's p0 needs
                    # the psAV banks back
                    reserve = 0 if last_u else AV_RESERVE
                    if u >= 1:
                        if len(avq) > AVQ_CAP:
                            max_pops = 3
                        elif len(avq) > reserve:
                            max_pops = 2
                        else:
                            max_pops = 0
                    else:
                        max_pops = 0
                    budget = PAIR_BUDGET - HALF_BUDGET
                    pops = 0
                    while pops < max_pops and budget >= 4:
                        got, was_p3 = pop_av()
                        if not got:
                            break
                        pops += 1
                        budget -= got
                        if was_p3 and not last_u:
                            pops = max_pops  # normalize-chain window
                    while budget > 0:
                        item = next_filler()
                        if item is None:
                            if pops < max(max_pops, 1) and len(avq) > reserve:
                                got, was_p3 = pop_av()
                                if got:
                                    budget -= got
                                    if was_p3 and not last_u:
                                        pops = max(max_pops, 1)
                                    continue
                            break
                        budget -= item[0]
                        item[1]()

            # drain: remaining fillers, then trailing AV groups; after each
            # unit-final pop, pad with dummy MMs (into a fresh psP tile —
            # wacc's psAV slot is recycled by now) to cover the psAV recycle
            def drain_pad():
                acc = psP.tile([128, 512], F32, tag="psP")
                nc.tensor.matmul(acc[:, 0:128], wrm[:], wrm[:],
                                 start=True, stop=True)
                nc.tensor.matmul(acc[:, 128:256], wrm[:], wrm[:],
                                 start=True, stop=True)
                nc.tensor.matmul(acc[:, 256:384], wrm[:], wrm[:],
                                 start=True, stop=True)

            while True:
                item = next_filler()
                if item is None:
                    break
                item[1]()
            while avq:
                au, ap_ = avq.pop(0)
                av_ops(au, ap_, pairs_of[au])
                if ap_ == NPAIR - 1 and avq:
                    drain_pad()

    nc.finalize()
    return nc


def _get_nc():
    if "nc" not in _CACHE:
        _CACHE["nc"] = _build()
    return _CACHE["nc"]


def kernel(x, tokens, Wq, bq, Wk, bk, Wv, bv):
    x = np.asarray(x, dtype=np.float32)
    tokens = np.asarray(tokens, dtype=np.float32)
    Wq = np.asarray(Wq, dtype=np.float32)
    Wk = np.asarray(Wk, dtype=np.float32)
    Wv = np.asarray(Wv, dtype=np.float32)
    bq = np.asarray(bq, dtype=np.float32)
    bk = np.asarray(bk, dtype=np.float32)
    bv = np.asarray(bv, dtype=np.float32)

    bf16 = ml_dtypes.bfloat16
    in_maps = []
    for c in range(NCORES):
        b, g = divmod(c, 2)
        rows = slice(g * FPG, (g + 1) * FPG)
        tq = tokens[b, 0] @ Wq[rows].T + 2.0 * bq[rows]   # [512]
        tk = tokens[b, 0] @ Wk[rows].T + 2.0 * bk[rows]

        def packw(aT):
            # [D, C] -> [128, NKC, C] partition-major
            return np.ascontiguousarray(
                aT.reshape(NKC, 128, aT.shape[1]).transpose(1, 0, 2)
            ).astype(bf16)

        xTb = x[b].T.reshape(NKC, 128, S)   # [kc, p, s]
        wqT = Wq[rows].T
        wkT = Wk[rows].T
        m = {"wv": packw(Wv[rows].T)}
        qadd = np.ascontiguousarray(
            (tq / 8.0).reshape(NFT, 128).T).astype(bf16).astype(np.float32).view(bf16)
        kadd = np.ascontiguousarray(
            tk.reshape(NFT, 128).T).astype(bf16).astype(np.float32).view(bf16)
        for f in range(NFT):
            m[f"wq{f}"] = packw(wqT[:, f * 128:(f + 1) * 128])
            wk_f = packw(wkT[:, f * 128:(f + 1) * 128]).reshape(128, NKC * 128)
            if f == 0:
                # qadd/kadd ride in the last 8 bf16 columns of each wk0 row
                wk_f = np.ascontiguousarray(
                    np.concatenate([wk_f, qadd, kadd], axis=1))
            m[f"wk{f}"] = wk_f
        for ci, cl in enumerate("ABCD"):
            xp = xTb[2 * ci:2 * ci + 2].transpose(1, 0, 2)  # [128, 2, 1024]
            m[f"x{cl}0"] = np.ascontiguousarray(xp[:, :, 0:512]).astype(bf16)
            m[f"x{cl}1"] = np.ascontiguousarray(xp[:, :, 512:1024]).astype(bf16)
        in_maps.append(m)

    nc = _get_nc()
    trace = bool(int(os.environ.get("KERNEL_TRACE", "0")))
    res = run_bass_kernel_spmd(nc, in_maps, core_ids=list(range(NCORES)), trace=trace)
    if trace:
        _CACHE["last_results"] = res

    y = np.empty((B, S, D), dtype=np.float32)
    for c in range(NCORES):
        b, g = divmod(c, 2)
        yT = np.asarray(res.results[c]["yT"], dtype=np.float32)  # [4, 128, 1024]
        y[b, :, g * FPG:(g + 1) * FPG] = yT.reshape(FPG, S).T
    y += bv[None, None, :]
    return y


# revision 40
# speedup vs baseline: 1.0056x; 1.0056x over previous
"""Guide-token attention kernel for Trainium2 (8 NeuronCores).

Module: y[b] = softmax(((Q+tQ) @ (K+tK)^T)/sqrt(hd)) @ V  per head, where
  Q = x @ Wq^T + bq, K = x @ Wk^T + bk, V = x @ Wv^T + bv,
  tQ/tK are projections of a per-batch guide token (broadcast over seq).

Shapes: x [4, 1024, 1024], tokens [4, 1, 1024], W* [1024, 1024], b* [1024].
H=16 heads, hd=64.

Sharding: 8 cores = 4 batches x 2 head-groups (8 heads each); weights
column-sharded per head group; each core sees one batch -> no cross-core
communication.

Design (v8, from v7 + trace-driven fixes):
  - PE is the bottleneck: 384 effective N=512 slots x 216 ns ~= 83 us.
    v7 trace: steady-state already hits 216 ns/slot; losses were (a) ACT
    head-of-line-blocked until 21.9us by input-DMA posts on the Scalar
    queue, (b) HAM cold until 20.3us, (c) end-phase AV backlog draining
    after ACT finished, with HAM re-throttle + psAV recycle blocked on the
    normalize chain, (d) 5.1us final normalize chain.
  - Input DMA posts now ride the Sync + GpSimd queues; the Scalar queue
    runs exps only (plus phase-0 evictions + early table load).
  - Half-pair score pipeline: one [128, 2, 512] PSUM tile (A_j B_j row
    pair) + one exp per kt, psA bufs=2 -> a true rolling double buffer.
    scores(N+1) overlap exp(N); the v7/v8.0 schemes allocated both bufs
    per pair, serializing exp -> scores -> exp (~0.9us/pair lost).
  - 28 N=128 warmup matmuls upfront (wrm memset on GpSimd before its DMA
    posts) bridge the preamble->DMA-ready window; HAM flips to 2.4 GHz at
    ~10.7us and the real MM stream starts warm instead of at 427 ns/MM.
  - AV pops start at u>=1, keep backlog ~2, and break after a p3 pop so
    the unit's normalize chain gets a pair-window before the next unit's
    p0 alloc needs the psAV banks.
  - Normalize: reciprocal straight from the PSUM den row, av copied
    PSUM->SBUF (frees the bank ~2.4us earlier), GpSimd partition-bcast,
    one DVE multiply, per-(hp,qb) 128KB output flush.
"""

import os

import numpy as np
import ml_dtypes

import concourse.bass as bass
import concourse.tile as tile
from concourse import bacc
from concourse import mybir
from concourse.bass_utils import run_bass_kernel_spmd

B = 4
S = 1024
D = 1024
H = 16
HD = 64
NCORES = 8
FPG = 512          # features per head-group (8 heads * 64)
NKC = D // 128     # contraction chunks for projections
NFT = FPG // 128   # feature tiles per group
NST = S // 128     # sequence tiles
NQB = S // 512     # 512-wide query blocks
HPG = 8            # heads per group
NPAIR = NST // 2   # kt pairs per unit

BF16 = mybir.dt.bfloat16
F32 = mybir.dt.float32

PAIR_BUDGET = 10   # non-score PE slots emitted per score pair
AVQ_CAP = 10       # force AV pops above this backlog (bounds probs pool)
AV_RESERVE = 2     # AV groups held back as PE anti-starvation reserve

_CACHE = {}


def _build():
    nc = bacc.Bacc()

    # ---- DRAM inputs ----
    # sync queue:   xA0 xB0 qadd kadd wk0 xA1 xB1 wk1       (+ yT out later)
    # gpsimd queue: wq0 xC0 xD0 wq1 xC1 xD1 wv wq2 wk2 wq3 wk3
    xd = {}
    for cname in ("xA0", "xB0", "xC0", "xD0", "xA1", "xB1", "xC1", "xD1"):
        xd[cname] = nc.declare_dram_parameter(cname, [128, 2, 512], BF16, isOutput=False)
    wqd = [nc.declare_dram_parameter(f"wq{f}", [128, NKC, 128], BF16, isOutput=False)
           for f in range(NFT)]
    # wk0 carries qadd+kadd (fp32 bit-packed into bf16 column pairs) in
    # the last 16 columns of each row — a standalone [128,4] fp32 transfer
    # is 128 16-byte descriptors and stalls the HW DGE ring for ~3us
    # (descriptor-rate-bound)
    wkd = [nc.declare_dram_parameter(
               f"wk{f}", [128, NKC * 128 + (16 if f == 0 else 0)], BF16,
               isOutput=False)
           for f in range(NFT)]
    wvd = nc.declare_dram_parameter("wv", [128, NKC, FPG], BF16, isOutput=False)
    yTd = nc.declare_dram_parameter("yT", [NFT, 128, S], BF16, isOutput=True)

    with tile.TileContext(nc) as tc:
        with (
            tc.tile_pool(name="persist", bufs=1) as persist,
            tc.tile_pool(name="probs", bufs=24) as probs_pool,
            tc.tile_pool(name="norm", bufs=4) as norm_pool,
            tc.tile_pool(name="avsb", bufs=4) as avsb_pool,
            tc.tile_pool(name="psP", bufs=2, space=bass.MemorySpace.PSUM) as psP,
            tc.tile_pool(name="psA", bufs=2, space=bass.MemorySpace.PSUM) as psA,
            tc.tile_pool(name="psAV", bufs=2, space=bass.MemorySpace.PSUM) as psAV,
        ):
            # ---- persistent SBUF tensors (chunked to DMA granularity) ----
            xts = {(c, h): persist.tile([128, 2, 512], BF16, name=f"xt{c}{h}")
                   for c in range(4) for h in range(2)}
            wq_sb = [persist.tile([128, NKC, 128], BF16, name=f"wqs{f}")
                     for f in range(NFT)]
            wk_sb = [persist.tile([128, NKC * 128 + (16 if f == 0 else 0)],
                                  BF16, name=f"wks{f}")
                     for f in range(NFT)]
            wv_sb = persist.tile([128, NKC, FPG], BF16)
            cq = [persist.tile([128, S], BF16, name=f"cq{i}") for i in range(NFT)]
            ck = [persist.tile([128, S], BF16, name=f"ck{i}") for i in range(NFT)]
            vts = [persist.tile([128, HPG, HD + 1], BF16, name=f"vt{i}")
                   for i in range(NST)]
            yt = persist.tile([128, NFT, S], BF16)
            wrm = persist.tile([128, 128], BF16)
            scr = persist.tile([1, 4], F32)

            # warmup source memset FIRST on the (otherwise idle) gpsimd
            # queue so warmup matmuls can start right after the preamble
            nc.gpsimd.memset(wrm[:], 0.0)
            nc.gpsimd.memset(scr[:], 0.0)

            # ---- input DMAs on the two HW DGE rings (SP + Act only — the
            # gpsimd ring is software-DGE: ~5us post-to-land latency and an
            # expensive teardown drain).  Scalar takes only the 5 earliest
            # posts (ring-fresh, done by ~10.5us) so exps never queue behind
            # a ring-flow-control wait; sync takes the rest.  ----
            nc.scalar.dma_start(out=wq_sb[0][:], in_=wqd[0][:])
            nc.scalar.dma_start(out=wk_sb[0][:], in_=wkd[0][:])
            nc.scalar.dma_start(out=xts[(2, 0)][:], in_=xd["xC0"][:])
            nc.scalar.dma_start(out=xts[(3, 0)][:], in_=xd["xD0"][:])
            nc.scalar.dma_start(out=wq_sb[1][:], in_=wqd[1][:])
            nc.sync.dma_start(out=xts[(0, 0)][:], in_=xd["xA0"][:])
            nc.sync.dma_start(out=xts[(1, 0)][:], in_=xd["xB0"][:])
            nc.sync.dma_start(out=xts[(0, 1)][:], in_=xd["xA1"][:])
            nc.sync.dma_start(out=xts[(1, 1)][:], in_=xd["xB1"][:])
            nc.sync.dma_start(out=wk_sb[1][:], in_=wkd[1][:])
            nc.sync.dma_start(out=wv_sb[:], in_=wvd[:])
            nc.sync.dma_start(out=xts[(2, 1)][:], in_=xd["xC1"][:])
            nc.sync.dma_start(out=xts[(3, 1)][:], in_=xd["xD1"][:])
            nc.sync.dma_start(out=wq_sb[2][:], in_=wqd[2][:])
            nc.sync.dma_start(out=wk_sb[2][:], in_=wkd[2][:])
            nc.sync.dma_start(out=wq_sb[3][:], in_=wqd[3][:])
            nc.sync.dma_start(out=wk_sb[3][:], in_=wkd[3][:])

            # ones columns for the AV denominator rows
            for st in range(NST):
                nc.vector.memset(vts[st][:, :, HD:HD + 1], 1.0)

            # trigger the ACT table load ASAP so the first exp doesn't pay
            # it; uses a dedicated scratch so warmup matmuls (which read
            # wrm) don't serialize behind this ACT op
            nc.scalar.activation(
                out=scr[0:1, 0:1], in_=scr[0:1, 0:1],
                func=mybir.ActivationFunctionType.Identity,
            )

            # ---- HAM warmup machinery (N=128: 8x cheaper queue time) ----
            wacc = psAV.tile([128, 512], F32, tag="psAV")
            warm_left = [52]

            def warm_mm():
                if warm_left[0] > 0:
                    warm_left[0] -= 1
                    nc.tensor.matmul(
                        wacc[:, 0:128], wrm[:], wrm[:], start=True, stop=True
                    )

            # bridge preamble-end (~7.4us) to first-chunk-landed (~12.3us:
            # HW-DGE ring start ~2.5us + ~1.1us per 256KB chunk) so the PE
            # never idles long enough to reset the HAM activity window —
            # phase-0 then runs entirely at 2.4 GHz
            for _ in range(38):
                warm_mm()

            # ---- projection building blocks ----
            v_done = [0]      # V groups fully emitted (gates AV emission)
            qk_done = set()   # (which, ft, sb) evictions emitted

            def qk_group(which, ft, sb, evict_on_act=False):
                """QT/KT [128 feat, 512 q] accumulated over D chunks, evicted
                to bf16 with the guide-token add (+1/8 scale folded into Q).
                Yields (slots, op)."""
                base = NKC * 128
                if which == "q":
                    scale, dst = 0.125, cq[ft]
                    w_col = lambda kc: wq_sb[ft][:, kc, :]
                    bias = wk_sb[0][:, base + 2 * ft:base + 2 * ft + 2]
                else:
                    scale, dst = 1.0, ck[ft]
                    w_col = lambda kc: wk_sb[ft][:, kc * 128:(kc + 1) * 128]
                    bias = wk_sb[0][:, base + 8 + 2 * ft:base + 8 + 2 * ft + 2]
                bias = bias.bitcast(F32)
                acc = psP.tile([128, 512], F32, tag="psP")
                for kc in range(NKC):
                    yield 1, lambda kc=kc, acc=acc: nc.tensor.matmul(
                        acc[:],
                        w_col(kc),
                        xts[(kc // 2, sb)][:, kc % 2, :],
                        start=(kc == 0),
                        stop=(kc == NKC - 1),
                    )

                def evict(acc=acc, bias=bias):
                    if evict_on_act:
                        # ScalarE is idle before its first exp: out =
                        # Identity(acc*scale + bias) == the same eviction,
                        # off the (busier) VectorE queue
                        nc.scalar.activation(
                            out=dst[:, sb * 512:(sb + 1) * 512],
                            in_=acc[:],
                            func=mybir.ActivationFunctionType.Identity,
                            bias=bias,
                            scale=scale,
                        )
                    else:
                        nc.vector.tensor_scalar(
                            out=dst[:, sb * 512:(sb + 1) * 512],
                            in0=acc[:],
                            scalar1=scale,
                            scalar2=bias,
                            op0=mybir.AluOpType.mult,
                            op1=mybir.AluOpType.add,
                        )
                    qk_done.add((which, ft, sb))

                yield 0, evict

            def v_group(st):
                """V [128 seq, 512 feat] natural layout, strided into vts."""
                acc = psP.tile([128, 512], F32, tag="psP")
                for kc in range(NKC):
                    yield 1, lambda kc=kc, acc=acc: nc.tensor.matmul(
                        acc[:],
                        xts[(kc // 2, st // 4)][:, kc % 2,
                                                (st % 4) * 128:(st % 4 + 1) * 128],
                        wv_sb[:, kc, :],
                        start=(kc == 0),
                        stop=(kc == NKC - 1),
                    )

                def evict(acc=acc):
                    nc.vector.tensor_copy(out=vts[st][:, :, 0:HD], in_=acc[:])
                    v_done[0] += 1

                yield 0, evict

            # keyed filler groups: budget flow follows FILLER_ORDER, but
            # pair-readiness drains pull exactly the group they need (groups
            # are self-contained, so out-of-order emission is dep-safe)
            FILLER_ORDER = (
                [("k", 0, 1), ("q", 0, 1), ("q", 1, 0), ("k", 1, 0),
                 ("k", 1, 1), ("v", 0), ("v", 1), ("v", 2), ("v", 3),
                 ("q", 1, 1)]
                + [("v", st) for st in range(4, NST)]
                + [("q", 2, 0), ("k", 2, 0), ("k", 2, 1), ("q", 2, 1),
                   ("q", 3, 0), ("k", 3, 0), ("k", 3, 1), ("q", 3, 1)]
            )
            fgens = {}
            for key in FILLER_ORDER:
                if key[0] == "v":
                    fgens[key] = v_group(key[1])
                else:
                    fgens[key] = qk_group(*key)

            def drain_group(key):
                g = fgens.get(key)
                if g is None:
                    return
                for _, op in g:
                    op()
                del fgens[key]

            def filler_stream():
                for key in FILLER_ORDER:
                    g = fgens.get(key)
                    if g is None:
                        continue
                    sentinel = object()
                    while True:
                        item = next(g, sentinel)
                        if item is sentinel:
                            break
                        yield item
                    if fgens.get(key) is g:
                        del fgens[key]

            # ---- AV + normalize ----
            av_tiles = {}

            def av_ops(u, p, pairs):
                hp, qb = UNITS[u]
                if p == 0:
                    av_tiles[u] = (
                        psAV.tile([HD + 1, 512], F32, tag="psAV", name=f"av{u}e"),
                        psAV.tile([HD + 1, 512], F32, tag="psAV", name=f"av{u}o"),
                    )
                av_e, av_o = av_tiles[u]
                last_u = u == len(UNITS) - 1
                dens = {}
                for j in range(2):
                    kt = 2 * p + j
                    prs = pairs[kt]
                    nc.tensor.matmul(
                        av_e[:], vts[kt][:, 2 * hp, :], prs[:, 0, :],
                        start=(kt == 0), stop=(kt == NST - 1),
                    )
                    if last_u and kt == NST - 1:
                        # den_e copy issues while the final o-head matmul
                        # still runs — shortens the tail chain
                        den_e = norm_pool.tile([1, 512], F32, tag="den",
                                               name="den_e_tail")
                        nc.vector.tensor_copy(
                            out=den_e[:], in_=av_e[HD:HD + 1, :])
                        dens[0] = den_e
                    nc.tensor.matmul(
                        av_o[:], vts[kt][:, 2 * hp + 1, :], prs[:, 1, :],
                        start=(kt == 0), stop=(kt == NST - 1),
                    )
                if p == NPAIR - 1:
                    qsl = slice(qb * 512, (qb + 1) * 512)
                    # note: custom-DVE ops (reciprocal_approx_fast) must
                    # read from base partition 0 on HW — the input partition
                    # offset is silently dropped (sim models it fine) — so
                    # the den row is first copied to partition 0 by a plain
                    # DVE copy.
                    if last_u:
                        # tail mode: nothing recycles these banks, so skip
                        # the av copies and multiply straight from PSUM;
                        # bcasts overlap the o-head's den/recip on DVE and
                        # the e-head's flush overlaps the o-head's multiply
                        recb_of = {}
                        for h_i, av in ((0, av_e), (1, av_o)):
                            den = dens.get(h_i)
                            if den is None:
                                den = norm_pool.tile([1, 512], F32, tag="den")
                                nc.vector.tensor_copy(
                                    out=den[:], in_=av[HD:HD + 1, :])
                            rec = norm_pool.tile([1, 512], F32, tag="rec")
                            nc.vector.reciprocal_approx_fast(
                                out=rec[:], in_=den[:])
                            recb = norm_pool.tile([HD, 512], F32, tag="recb")
                            nc.gpsimd.partition_broadcast(recb[:], rec[:])
                            recb_of[h_i] = recb
                        for h_i, av in ((0, av_e), (1, av_o)):
                            nc.vector.tensor_tensor(
                                out=yt[h_i * 64:h_i * 64 + 64, hp, qsl],
                                in0=av[0:HD, :],
                                in1=recb_of[h_i][:],
                                op=mybir.AluOpType.mult,
                            )
                            nc.sync.dma_start(
                                out=yTd[hp][h_i * 64:h_i * 64 + 64, qsl],
                                in_=yt[h_i * 64:h_i * 64 + 64, hp, qsl])
                    else:
                        for h_i, av in ((0, av_e), (1, av_o)):
                            # av copy FIRST: the psAV bank frees ~0.7us
                            # after the last AV matmul instead of after the
                            # whole normalize chain (kills the ~1.4us PE
                            # stall at every unit boundary)
                            sb_av = avsb_pool.tile([HD + 1, 512], F32,
                                                   tag="avsb")
                            nc.vector.tensor_copy(out=sb_av[:], in_=av[:])
                            den = norm_pool.tile([1, 512], F32, tag="den")
                            nc.vector.tensor_copy(
                                out=den[:], in_=sb_av[HD:HD + 1, :])
                            rec = norm_pool.tile([1, 512], F32, tag="rec")
                            nc.vector.reciprocal_approx_fast(
                                out=rec[:], in_=den[:])
                            recb = norm_pool.tile([HD, 512], F32, tag="recb")
                            nc.gpsimd.partition_broadcast(recb[:], rec[:])
                            nc.vector.tensor_tensor(
                                out=yt[h_i * 64:h_i * 64 + 64, hp, qsl],
                                in0=sb_av[0:HD, :],
                                in1=recb[:],
                                op=mybir.AluOpType.mult,
                            )
                        nc.sync.dma_start(out=yTd[hp][:, qsl],
                                          in_=yt[:, hp, qsl])
                    del av_tiles[u]

            # ---- phase 0: Q00 and K00 interleaved kc-wise so both track x
            # chunk arrival; warmups pad the DMA-bound stretch (HAM warm) ----
            g_q = qk_group("q", 0, 0, evict_on_act=True)
            g_k = qk_group("k", 0, 0, evict_on_act=True)
            for i in range(NKC + 1):
                for g in (g_q, g_k):
                    item = next(g, None)
                    if item is not None:
                        item[1]()
                warm_mm()
            # cover the eviction->first-score-pair latency
            for _ in range(3):
                warm_mm()

            UNITS = [(hp, qb) for hp in range(HPG // 2) for qb in range(NQB)]
            fillers = filler_stream()
            fillers_done = [False]
            avq = []
            pairs_of = {}

            def next_filler():
                item = next(fillers, None)
                if item is None:
                    fillers_done[0] = True
                    return None
                return item

            def pop_av():
                """Emit the oldest pending AV group if allowed; 4 PE slots.
                Returns (slots, was_last_group_of_unit)."""
                if not avq:
                    return 0, False
                au, ap_ = avq[0]
                if v_done[0] < 2 * ap_ + 2:
                    return 0, False
                avq.pop(0)
                av_ops(au, ap_, pairs_of[au])
                return 4, ap_ == NPAIR - 1

            def pair_ready(hp, qb, p):
                return ("q", hp, qb) in qk_done and ("k", hp, p // 2) in qk_done

            def emit_half(u, p, j):
                """One kt's scores (A/B row-tiled pair) + one exp.  With psA
                bufs=2 this forms a rolling double buffer: scores for half
                N+1 overlap exp N on ACT."""
                hp, qb = UNITS[u]
                qsl = slice(qb * 512, (qb + 1) * 512)
                kt = 2 * p + j
                ksl = slice(kt * 128, (kt + 1) * 128)
                sct = psA.tile([128, 2, 512], F32, tag="psA")
                nc.tensor.matmul(
                    sct[:, 0, :], ck[hp][0:64, ksl], cq[hp][0:64, qsl],
                    start=True, stop=True,
                )
                nc.tensor.matmul(
                    sct[:, 1, :], ck[hp][64:128, ksl], cq[hp][64:128, qsl],
                    start=True, stop=True,
                )
                prs = probs_pool.tile([128, 2, 512], BF16, tag="probs")
                nc.scalar.activation(
                    out=prs[:], in_=sct[:],
                    func=mybir.ActivationFunctionType.Exp,
                )
                pairs_of[u][kt] = prs
                if j == 1:
                    avq.append((u, p))

            HALF_BUDGET = PAIR_BUDGET // 2
            for u in range(len(UNITS)):
                pairs_of[u] = {}
                last_u = u == len(UNITS) - 1
                for p in range(NPAIR):
                    # correctness: projections this pair reads must be
                    # in-stream before its score matmuls (targeted pull)
                    hp_, qb_ = UNITS[u]
                    if ("q", hp_, qb_) not in qk_done:
                        drain_group(("q", hp_, qb_))
                    if ("k", hp_, p // 2) not in qk_done:
                        drain_group(("k", hp_, p // 2))
                    emit_half(u, p, 0)
                    # fillers keep PE busy while exp(kt0) runs
                    budget = HALF_BUDGET
                    while budget > 0:
                        item = next_filler()
                        if item is None:
                            break
                        budget -= item[0]
                        item[1]()
                    emit_half(u, p, 1)
                    # rate-match ACT: AV pops + projection fillers; after a
                    # unit-final (p3) pop, stop popping so its normalize
                    # chain gets a window before the next unit's p0 needs
                    # the psAV banks back
                    reserve = 0 if last_u else AV_RESERVE
                    if u >= 1:
                        if len(avq) > AVQ_CAP:
                            max_pops = 3
                        elif len(avq) > reserve:
                            max_pops = 2
                        else:
                            max_pops = 0
                    else:
                        max_pops = 0
                    budget = PAIR_BUDGET - HALF_BUDGET
                    pops = 0
                    while pops < max_pops and budget >= 4:
                        got, was_p3 = pop_av()
                        if not got:
                            break
                        pops += 1
                        budget -= got
                        if was_p3 and not last_u:
                            pops = max_pops  # normalize-chain window
                    while budget > 0:
                        item = next_filler()
                        if item is None:
                            if pops < max(max_pops, 1) and len(avq) > reserve:
                                got, was_p3 = pop_av()
                                if got:
                                    budget -= got
                                    if was_p3 and not last_u:
                                        pops = max(max_pops, 1)
                                    continue
                            break
                        budget -= item[0]
                        item[1]()

            # drain: remaining fillers, then trailing AV groups; after each
            # unit-final pop, pad with dummy MMs (into a fresh psP tile —
            # wacc's psAV slot is recycled by now) to cover the psAV recycle
            def drain_pad():
                acc = psP.tile([128, 512], F32, tag="psP")
                nc.tensor.matmul(acc[:, 0:128], wrm[:], wrm[:],
                                 start=True, stop=True)
                nc.tensor.matmul(acc[:, 128:256], wrm[:], wrm[:],
                                 start=True, stop=True)
                nc.tensor.matmul(acc[:, 256:384], wrm[:], wrm[:],
                                 start=True, stop=True)

            while True:
                item = next_filler()
                if item is None:
                    break
                item[1]()
            while avq:
                au, ap_ = avq.pop(0)
                av_ops(au, ap_, pairs_of[au])
                if ap_ == NPAIR - 1 and avq:
                    drain_pad()

    nc.finalize()
    return nc


def _get_nc():
    if "nc" not in _CACHE:
        _CACHE["nc"] = _build()
    return _CACHE["nc"]


def kernel(x, tokens, Wq, bq, Wk, bk, Wv, bv):
    x = np.asarray(x, dtype=np.float32)
    tokens = np.asarray(tokens, dtype=np.float32)
    Wq = np.asarray(Wq, dtype=np.float32)
    Wk = np.asarray(Wk, dtype=np.float32)
    Wv = np.asarray(Wv, dtype=np.float32)
    bq = np.asarray(bq, dtype=np.float32)
    bk = np.asarray(bk, dtype=np.float32)
    bv = np.asarray(bv, dtype=np.float32)

    bf16 = ml_dtypes.bfloat16
    in_maps = []
    for c in range(NCORES):
        b, g = divmod(c, 2)
        rows = slice(g * FPG, (g + 1) * FPG)
        tq = tokens[b, 0] @ Wq[rows].T + 2.0 * bq[rows]   # [512]
        tk = tokens[b, 0] @ Wk[rows].T + 2.0 * bk[rows]

        def packw(aT):
            # [D, C] -> [128, NKC, C] partition-major
            return np.ascontiguousarray(
                aT.reshape(NKC, 128, aT.shape[1]).transpose(1, 0, 2)
            ).astype(bf16)

        xTb = x[b].T.reshape(NKC, 128, S)   # [kc, p, s]
        wqT = Wq[rows].T
        wkT = Wk[rows].T
        m = {"wv": packw(Wv[rows].T)}
        qadd = np.ascontiguousarray(
            (tq / 8.0).reshape(NFT, 128).T).astype(bf16).astype(np.float32).view(bf16)
        kadd = np.ascontiguousarray(
            tk.reshape(NFT, 128).T).astype(bf16).astype(np.float32).view(bf16)
        for f in range(NFT):
            m[f"wq{f}"] = packw(wqT[:, f * 128:(f + 1) * 128])
            wk_f = packw(wkT[:, f * 128:(f + 1) * 128]).reshape(128, NKC * 128)
            if f == 0:
                # qadd/kadd ride in the last 8 bf16 columns of each wk0 row
                wk_f = np.ascontiguousarray(
                    np.concatenate([wk_f, qadd, kadd], axis=1))
            m[f"wk{f}"] = wk_f
        for ci, cl in enumerate("ABCD"):
            xp = xTb[2 * ci:2 * ci + 2].transpose(1, 0, 2)  # [128, 2, 1024]
            m[f"x{cl}0"] = np.ascontiguousarray(xp[:, :, 0:512]).astype(bf16)
            m[f"x{cl}1"] = np.ascontiguousarray(xp[:, :, 512:1024]).astype(bf16)
        in_maps.append(m)

    nc = _get_nc()
    trace = bool(int(os.environ.get("KERNEL_TRACE", "0")))
    res = run_bass_kernel_spmd(nc, in_maps, core_ids=list(range(NCORES)), trace=trace)
    if trace:
        _CACHE["last_results"] = res

    y = np.empty((B, S, D), dtype=np.float32)
    for c in range(NCORES):
        b, g = divmod(c, 2)
        yT = np.asarray(res.results[c]["yT"], dtype=np.float32)  # [4, 128, 1024]
        y[b, :, g * FPG:(g + 1) * FPG] = yT.reshape(FPG, S).T
    y += bv[None, None, :]
    return y
